# revision 32
# baseline (speedup 1.0000x reference)
"""AttentionPerformer Trainium2 kernel (v3).

Data-parallel over batch B=8 -> one NeuronCore per batch element.

Numerics: fp16 on the exponent-critical path (x, Wk/Wq, k, q, k^2, q^2, w),
bf16 on the value path (kp, qp, v, kptv, G, qpn, output). fp16/bf16 both
stream the PE at 1 cycle/row. Validated rel_fro ~= 5e-3.

v3 changes over v2 (trace-driven):
  - PE warmup MM block at kernel start: the HAM clock gate otherwise keeps
    the PE at 1.2 GHz for the first ~30us (DMA-starved start never fills a
    busy window). ~34 dummy MMs warm it during the initial weight DMA.
  - wkq weight load collapsed 6 DMAs -> 2 (chunk0, chunks1-5) so the first
    kq matmul's semaphore wait covers only chunk0; small consts moved to
    the gpsimd DMA queue.
  - prm quad matmuls emitted interleaved across the two col-groups
    (tile_position (0,0)/(0,64)) so adjacent MMs run concurrently in
    different PE column groups (~2x on the prm block).
  - transpose PSUM slots packed into one bank (ring of 8) freeing a bank
    for triple-buffered pairK/pairQ psum.
  - interlude + pass 2 merged: ksum diag extracted first, qpn production
    overlaps kptv/G matmuls, y-GEMM processes two t-tiles per G-weight.

Structure (per core, per head h; heads paired (2p, 2p+1)):
  pass 1 (per 512-col tile of x^T):
    pairK/pairQ = Wkq-block^T @ x^T            (12x [128,512] psum, fp16 in)
    kk = f16(pairK + bk); kksq = (pairK+bk)^2  (DVE tsa move + Scalar Square)
    [pk_h1; pk_h2] = blockdiag(w_h1, w_h2) @ kk + blockdiag(-.5) @ kksq
      -> quad psumEK/psumEQ [128,512] via tile_position, 2 MMs each
    kp = exp(psumEK) [128=4 heads x 32m, 512]   (1 act per quad)
    qp -> qp_pack    [128, 3, 4096] resident    (1 act per quad)
    kp transpose [128,128] chunks -> kpn [128t, 128m]
    xkacc[m, g, c(768)+ksum(1)] += kpn_g^T @ [xN | 1]   (PSUM-resident)
  interlude/pass 2:
    ksumdiag from xkacc col C; kptv = Wv^T @ XKP; G = kptv^T @ PwT
    D = ksumdiag^T @ qp_pack; rec = 1/(D+eps); bcast via indicator MM
    qpn = qp * rec_bcast   (pipelined 2 tiles ahead of the y-GEMM)
    yT[c'-chunk, t] = sum_g G_g-block^T @ qpn_g  (+proj bias) -> DMA out
"""
import sys
sys.path.insert(0, '/opt/trn_rl_repo')

import numpy as np
import ml_dtypes

B, N, C = 8, 4096, 768
H, HD, M = 12, 64, 32
T = 512                 # t-tile size
NT = N // T             # 8 tiles
NP = 6                  # head pairs
EPS_EFF = float(M) * 1e-8
WARMUP = 40             # dummy PE MMs (N=128) to warm the HAM clock gate
PRM_MODE = "P"          # 'P': per-element has_written clear; 'W': whole-bank

_CACHE = {}
TRACE = False
LAST_EXEC_NS = None


def _build():
    import concourse.bass as bass
    import concourse.tile as tile
    from concourse import bacc, mybir

    f32 = mybir.dt.float32
    f16 = mybir.dt.float16
    bf16 = mybir.dt.bfloat16
    ADD = mybir.AluOpType.add
    MULT = mybir.AluOpType.mult
    EXP = mybir.ActivationFunctionType.Exp
    SQUARE = mybir.ActivationFunctionType.Square
    IDENT = mybir.ActivationFunctionType.Identity

    nc = bacc.Bacc()

    xT = nc.dram_tensor("xT", [C, N], f16, kind="ExternalInput")
    wkq = nc.dram_tensor("wkq", [6, 128, 2 * C], f16, kind="ExternalInput")
    wv = nc.dram_tensor("wv", [C, C], bf16, kind="ExternalInput")
    # xN carries a baked ones column (col C) for the ksum row of XKP
    xN = nc.dram_tensor("xN", [N, C + 1], bf16, kind="ExternalInput")
    prmw = nc.dram_tensor("prmw", [128, NP * 64], f16, kind="ExternalInput")
    nhalf = nc.dram_tensor("nhalf", [128, 64], f16, kind="ExternalInput")
    kqb = nc.dram_tensor("kqb", [128, 2 * NP], f32, kind="ExternalInput")
    pwT = nc.dram_tensor("pwT", [64, H * C], bf16, kind="ExternalInput")
    pb = nc.dram_tensor("pb", [128, 6], f32, kind="ExternalInput")
    identb = nc.dram_tensor("identb", [128, 128], bf16, kind="ExternalInput")
    ind12 = nc.dram_tensor("ind12", [12, 3 * 128], bf16,
                           kind="ExternalInput")
    yT = nc.dram_tensor("yT", [C, N], bf16, kind="ExternalOutput")
    wuout = nc.dram_tensor("wuout", [128, 1], f32, kind="ExternalOutput")

    with tile.TileContext(nc) as tc:
        import contextlib
        with contextlib.ExitStack() as ctx:
            const = ctx.enter_context(tc.tile_pool(name="const", bufs=1))

            # ---- PE warmup: dummy MMs so the HAM clock gate reaches
            # K=8/8 while the initial weight DMAs stream ----
            wu_sb = const.tile([128, 128], f16, tag="wu")
            nc.vector.memset(wu_sb, 0.0)
            wu_keep = const.tile([128, 1], f32, tag="wukeep")
            with tc.tile_pool(name="ps_wu", bufs=1, space="PSUM") as ps_wu:
                wups = ps_wu.tile([128, 128], f32, tag="wups")
                for i in range(WARMUP):
                    nc.tensor.matmul(wups, wu_sb, wu_sb,
                                     start=(i == 0), stop=(i == WARMUP - 1))
                nc.vector.tensor_copy(wu_keep, wups[:, 0:1])

            # ---- resident constants ----
            # wkq per-chunk DMAs split across two queues so each chunk's
            # semaphore releases its MMs as it lands and the weight load
            # gets ~2/3 of HBM bandwidth at startup
            wkq_sb = const.tile([128, 6, 2 * C], f16, tag="wkq")
            for c in range(6):
                q = nc.scalar if c % 2 == 0 else nc.gpsimd
                q.dma_start(wkq_sb[:, c, :], wkq[c, :, :])
            wv_sb = []
            for c in range(6):
                t_ = const.tile([128, C], bf16, tag=f"wv{c}")
                wv_sb.append(t_)
            pwT_sb = const.tile([64, H, C], bf16, tag="pwT")
            prmw_sb = const.tile([128, NP, 64], f16, tag="prmw")
            nc.gpsimd.dma_start(prmw_sb, prmw[:].rearrange(
                "p (np w) -> p np w", np=NP))
            nhalf_sb = const.tile([128, 64], f16, tag="nhalf")
            nc.gpsimd.dma_start(nhalf_sb, nhalf[:])
            kqb_sb = const.tile([128, 2 * NP], f32, tag="kqb")
            nc.gpsimd.dma_start(kqb_sb, kqb[:])
            pb_sb = const.tile([128, 6], f32, tag="pb")
            ident_sb = const.tile([128, 128], bf16, tag="identb")
            nc.gpsimd.dma_start(ident_sb, identb[:])
            ind12_sb = const.tile([12, 3, 128], bf16, tag="ind12")

            # resident state
            qp_pack = const.tile([128, 3, N], bf16, tag="qp_pack")
            # XKP^T accumulator: [m(4x32 of quad g), g, c(768)+ksum(1)]
            # XKP[m, c] = sum_t kp[t, m] x[t, c];  col 768 (ones) = ksum
            xkacc = const.tile([128, 3, C + 1], f32, tag="xkacc")
            nc.vector.memset(xkacc, 0.0)
            xkb = const.tile([128, 3, C + 1], bf16, tag="xkb")
            xkp_sb = const.tile([128, 6, 3 * 128], bf16, tag="xkp_sb")

            # ================= PASS 1 =================
            with tc.tile_pool(name="xt", bufs=3) as xtp, \
                 tc.tile_pool(name="xnat", bufs=8) as xnp, \
                 tc.tile_pool(name="kkq", bufs=6) as kkp, \
                 tc.tile_pool(name="sqq", bufs=6) as sqp, \
                 tc.tile_pool(name="kpt", bufs=4) as kptp, \
                 tc.tile_pool(name="kpn", bufs=4) as kpnp, \
                 tc.tile_pool(name="ps_kq", bufs=3, space="PSUM") as ps_kq, \
                 tc.tile_pool(name="ps_x", bufs=1, space="PSUM") as ps_x, \
                 tc.tile_pool(name="ps_e", bufs=2, space="PSUM") as ps_e, \
                 tc.tile_pool(name="ps_tr", bufs=1, space="PSUM") as ps_tr:

                # one bank holds a ring of 8 [128,128]bf16 transpose slots
                ptr_ring = ps_tr.tile([128, 8, 128], bf16, tag="ptrring")
                tr_slot = [0]

                def load_xn(itn):
                    # xn rides the scalar DMA queue: the sync queue is
                    # saturated with xt and a late xn stalls the xkacc add
                    out = []
                    tn = itn * T
                    for j in range(4):
                        xn = xnp.tile([128, C + 1], bf16, tag="xnat")
                        nc.scalar.dma_start(
                            xn, xN[tn + j * 128:tn + (j + 1) * 128, :])
                        out.append(xn)
                    return out

                def early_xk(gq):
                    # quad gq's XKP is final once tile 7's accumulation
                    # lands: cast + transpose it here so the interlude's
                    # kptv/G chain starts with its inputs ready
                    nc.scalar.activation(xkb[:, gq, :], xkacc[:, gq, :],
                                         IDENT)
                    for cc in range(6):
                        slot = tr_slot[0]
                        tr_slot[0] = (slot + 1) % 8
                        ptr = ptr_ring[:, slot, :]
                        nc.tensor.transpose(
                            ptr, xkb[:, gq, cc * 128:(cc + 1) * 128],
                            ident_sb)
                        dst = xkp_sb[:, cc, gq * 128:(gq + 1) * 128]
                        if cc % 2 == 0:
                            nc.scalar.activation(dst, ptr, IDENT)
                        else:
                            nc.vector.tensor_copy(dst, ptr)

                xnats_next = load_xn(0)
                for it in range(NT):
                    t0 = it * T
                    if it == 1:
                        # interlude consts: issued here so the queue
                        # reaches them mid-pass-1 and the 2.4MB transfer
                        # hides under compute instead of gating the interlude
                        for c in range(6):
                            nc.sync.dma_start(
                                wv_sb[c], wv[c * 128:(c + 1) * 128, :])
                        nc.sync.dma_start(pwT_sb, pwT[:].rearrange(
                            "p (h c) -> p h c", h=H))
                        nc.sync.dma_start(pb_sb, pb[:])
                        nc.sync.dma_start(
                            ind12_sb,
                            ind12[:].rearrange("p (g w) -> p g w", g=3))
                    xt = xtp.tile([128, 6, T], f16, tag="xt")
                    for c in range(6):
                        nc.sync.dma_start(
                            xt[:, c, :], xT[c * 128:(c + 1) * 128, t0:t0 + T])
                    xnats = xnats_next

                    psEK = psEQ = None
                    pend = []       # per-pair prm MM descriptors
                    pend_quad = []

                    def flush_quad():
                        # emit the quad's 8 prm MMs interleaved across the
                        # two PE column groups so adjacent MMs overlap
                        assert len(pend) >= 2
                        pe, po = pend.pop(0), pend.pop(0)
                        # each entry: (EK, EQ, j2, kk, qq, kksq, qqsq, wslice)
                        def mm(ent, kind, start, stop):
                            EKq, EQq, j2, kk, qq, kksq, qqsq, wsl = ent
                            tp = (0, 64 * j2)
                            dstK = EKq[64 * j2:64 * j2 + 64, :]
                            dstQ = EQq[64 * j2:64 * j2 + 64, :]
                            if kind == "Kw":
                                nc.tensor.matmul(
                                    dstK, wsl, kk, start=start, stop=stop,
                                    tile_position=tp, skip_group_check=True)
                            elif kind == "Ksq":
                                nc.tensor.matmul(
                                    dstK, nhalf_sb, kksq, start=start,
                                    stop=stop, tile_position=tp,
                                    skip_group_check=True)
                            elif kind == "Qw":
                                nc.tensor.matmul(
                                    dstQ, wsl, qq, start=start, stop=stop,
                                    tile_position=tp, skip_group_check=True)
                            else:
                                nc.tensor.matmul(
                                    dstQ, nhalf_sb, qqsq, start=start,
                                    stop=stop, tile_position=tp,
                                    skip_group_check=True)
                        s2 = PRM_MODE == "P"   # second col-group start flag
                        mm(pe, "Kw", True, False)
                        mm(po, "Kw", s2, False)
                        mm(pe, "Ksq", False, True)
                        mm(po, "Ksq", False, True)
                        mm(pe, "Qw", True, False)
                        mm(po, "Qw", s2, False)
                        mm(pe, "Qsq", False, True)
                        mm(po, "Qsq", False, True)

                    def complete_quad(gq, EK, EQ):
                        # emit quad gq's deferred prm MMs, exps, transposes
                        # and XKP accumulation (one pair behind production,
                        # so independent kq MMs hide the move/square latency)
                        flush_quad()
                        kpt = kptp.tile([128, T], bf16, tag="kpt")
                        nc.scalar.activation(kpt, EK, EXP)
                        nc.scalar.activation(
                            qp_pack[:, gq, t0:t0 + T], EQ, EXP)
                        xk = ps_x.tile([128, C + 1], f32, tag="xkpt")
                        for j in range(4):
                            slot = tr_slot[0]
                            tr_slot[0] = (slot + 1) % 8
                            ptr = ptr_ring[:, slot, :]
                            nc.tensor.transpose(
                                ptr, kpt[:, j * 128:(j + 1) * 128],
                                ident_sb)
                            kpn = kpnp.tile([128, 128], bf16, tag="kpn")
                            nc.vector.tensor_copy(kpn, ptr)
                            nc.tensor.matmul(
                                xk[:, 0:512], kpn, xnats[j][:, 0:512],
                                start=(j == 0), stop=(j == 3))
                            nc.tensor.matmul(
                                xk[:, 512:C + 1], kpn,
                                xnats[j][:, 512:C + 1],
                                start=(j == 0), stop=(j == 3))
                        nc.vector.tensor_tensor(
                            xkacc[:, gq, :], xk, xkacc[:, gq, :], ADD)

                    for p in range(NP):
                        g, j2 = p // 2, p % 2
                        if p % 2 == 0:
                            psEK = ps_e.tile([128, T], f32, tag="psE")
                            psEQ = ps_e.tile([128, T], f32, tag="psE")
                        pairK = ps_kq.tile([128, T], f32, tag="pair")
                        pairQ = ps_kq.tile([128, T], f32, tag="pair")
                        for c in range(6):
                            nc.tensor.matmul(
                                pairK,
                                wkq_sb[:, c, p * 256:p * 256 + 128],
                                xt[:, c, :], start=(c == 0), stop=(c == 5))
                        for c in range(6):
                            nc.tensor.matmul(
                                pairQ,
                                wkq_sb[:, c, p * 256 + 128:p * 256 + 256],
                                xt[:, c, :], start=(c == 0), stop=(c == 5))
                        bK = kqb_sb[:, p:p + 1]
                        bQ = kqb_sb[:, NP + p:NP + p + 1]
                        kk = kkp.tile([128, T], f16, tag="kk")
                        qq = kkp.tile([128, T], f16, tag="kk")
                        nc.vector.tensor_scalar_add(kk, pairK, bK)
                        nc.vector.tensor_scalar_add(qq, pairQ, bQ)
                        kksq = sqp.tile([128, T], f16, tag="sq")
                        qqsq = sqp.tile([128, T], f16, tag="sq")
                        nc.scalar.activation(kksq, pairK, SQUARE, bias=bK)
                        nc.scalar.activation(qqsq, pairQ, SQUARE, bias=bQ)
                        pend.append((psEK, psEQ, j2, kk, qq, kksq, qqsq,
                                     prmw_sb[:, p, :]))
                        if p % 2 == 1:
                            pend_quad.append((g, psEK, psEQ))
                        if pend_quad and (p % 2 == 0 or p == NP - 1):
                            gq_done = pend_quad[0][0]
                            complete_quad(*pend_quad.pop(0))
                            if it == NT - 1:
                                early_xk(gq_done)
                    if it + 1 < NT:
                        xnats_next = load_xn(it + 1)

            # ========== INTERLUDE + PASS 2 (merged) ==========
            # ksum diag extracted first; then ALL tiles' D -> 1/(D+eps)
            # chains run upfront (dense PE work through the interlude, and
            # the bcast matmuls later never wait on the vector chain).
            eps_sb = const.tile([12, 1], f32, tag="eps")
            nc.vector.memset(eps_sb, EPS_EFF)
            ksc = const.tile([128, 3, 1], bf16, tag="ksc")
            nc.scalar.activation(ksc, xkacc[:, :, C:C + 1], IDENT)
            # ksumdiag[r, g, h] = ksum_quadg[r] iff h == 4g + r//32
            # (separate 12-col diag per quad: the D matmul accumulates the
            # three quads into one [12, T] psum)
            ksumdiag = const.tile([128, 3, 12], bf16, tag="ksumdiag")
            nc.vector.memset(ksumdiag, 0.0)
            for g in range(3):
                for gi in range(4):
                    nc.vector.tensor_copy(
                        ksumdiag[32 * gi:32 * (gi + 1), g,
                                 4 * g + gi:4 * g + gi + 1],
                        ksc[32 * gi:32 * (gi + 1), g, :])

            G_sb = []
            recs = []
            with tc.tile_pool(name="qpn", bufs=12) as qpnp, \
                 tc.tile_pool(name="rcs", bufs=10) as rcp, \
                 tc.tile_pool(name="rec", bufs=8) as recp, \
                 tc.tile_pool(name="yo", bufs=6) as yop, \
                 tc.tile_pool(name="ps_d", bufs=2, space="PSUM") as ps_d, \
                 tc.tile_pool(name="ps_b", bufs=2, space="PSUM") as ps_b:

                def produce_rec(itq):
                    # D -> 1/(D+eps) chain for one t-tile
                    tq = itq * T
                    dps = ps_d.tile([12, T], f32, tag="dps")
                    for g in range(3):
                        nc.tensor.matmul(
                            dps, ksumdiag[:, g, :],
                            qp_pack[:, g, tq:tq + T],
                            start=(g == 0), stop=(g == 2))
                    dpe = rcp.tile([12, T], f32, tag="dpe")
                    nc.scalar.activation(dpe, dps, IDENT, bias=eps_sb)
                    rec32 = rcp.tile([12, T], f32, tag="rec32")
                    nc.vector.reciprocal_approx_fast(out=rec32, in_=dpe)
                    rec = recp.tile([12, T], bf16, tag="rec")
                    nc.scalar.activation(rec, rec32, IDENT)
                    recs.append(rec)

                def produce_qpn(itq, rec):
                    # bcast + multiply only; rec is precomputed
                    tq = itq * T
                    out = []
                    for g in range(3):
                        bcast = ps_b.tile([128, T], f32, tag="bcast")
                        nc.tensor.matmul(bcast, ind12_sb[:, g, :],
                                         rec, start=True, stop=True)
                        qpn = qpnp.tile([128, T], bf16, tag="qpn")
                        nc.vector.tensor_tensor(
                            qpn, qp_pack[:, g, tq:tq + T], bcast, MULT)
                        out.append(qpn)
                    return out

                with tc.tile_pool(name="ps_g", bufs=2, space="PSUM") as \
                     ps_g, \
                     tc.tile_pool(name="ps_kv", bufs=1, space="PSUM") as \
                     ps_kv:
                    # first two rec chains only; the G-chain's vector/scalar
                    # work stays unblocked, the rest pipeline with the
                    # y-stream
                    produce_rec(0)
                    produce_rec(1)
                    kptv_ps = ps_kv.tile([64, H, M], f32, tag="kptv")
                    kptv_sb = const.tile([64, H, M], bf16, tag="kptv_sb")
                    for g in range(3):
                        for h in range(4 * g, 4 * g + 4):
                            gi = h % 4
                            for cc in range(6):
                                nc.tensor.matmul(
                                    kptv_ps[:, h, :],
                                    wv_sb[cc][:, h * 64:(h + 1) * 64],
                                    xkp_sb[:, cc,
                                           g * 128 + 32 * gi:
                                           g * 128 + 32 * (gi + 1)],
                                    start=(h == 4 * g and cc == 0),
                                    stop=(h == 4 * g + 3 and cc == 5))
                        nc.scalar.activation(kptv_sb[:, 4 * g:4 * g + 4, :],
                                             kptv_ps[:, 4 * g:4 * g + 4, :],
                                             IDENT)
                        gt = const.tile([128, C], bf16, tag=f"G{g}")
                        for half in range(2):
                            gps = ps_g.tile([128, 384], f32, tag="gps")
                            for gi in range(4):
                                h = 4 * g + gi
                                nc.tensor.matmul(
                                    gps[32 * gi:32 * (gi + 1), :],
                                    kptv_sb[:, h, :],
                                    pwT_sb[:, h,
                                           384 * half:384 * (half + 1)],
                                    start=True, stop=True,
                                    tile_position=(0, 32 * gi))
                            nc.vector.tensor_copy(
                                gt[:, 384 * half:384 * (half + 1)], gps)
                        G_sb.append(gt)
                    produce_rec(2)
                    produce_rec(3)

                # ---- y-GEMM stream: 2 t-tiles per G weight block;
                # qpn (bcast+mult) produced one tile-pair ahead, rec
                # chains two pairs ahead ----
                with tc.tile_pool(name="ps_y", bufs=4, space="PSUM") as \
                     ps_y:
                    qpn_next = [produce_qpn(0, recs[0]),
                                produce_qpn(1, recs[1])]
                    for tp2 in range(NT // 2):
                        it0, it1 = 2 * tp2, 2 * tp2 + 1
                        for r in (it1 + 3, it1 + 4):
                            if r < NT:
                                produce_rec(r)
                        qa = qpn_next.pop(0)
                        qb = qpn_next.pop(0)
                        if it1 + 1 < NT:
                            qpn_next.append(
                                produce_qpn(it1 + 1, recs[it1 + 1]))
                        if it1 + 2 < NT:
                            qpn_next.append(
                                produce_qpn(it1 + 2, recs[it1 + 2]))
                        for i2 in range(6):
                            ypsa = ps_y.tile([128, T], f32, tag="yps")
                            ypsb = ps_y.tile([128, T], f32, tag="yps")
                            for g in range(3):
                                gsl = G_sb[g][:, i2 * 128:(i2 + 1) * 128]
                                nc.tensor.matmul(
                                    ypsa, gsl, qa[g],
                                    start=(g == 0), stop=(g == 2))
                                nc.tensor.matmul(
                                    ypsb, gsl, qb[g],
                                    start=(g == 0), stop=(g == 2))
                            # both tiles land in one contiguous yT slab so
                            # a single DMA moves them; out-DMAs alternate
                            # sync/gpsimd to double issue throughput
                            yo2 = yop.tile([128, 2, T], bf16, tag="yo")
                            nc.scalar.activation(yo2[:, 0, :], ypsa, IDENT,
                                                 bias=pb_sb[:, i2:i2 + 1])
                            nc.vector.tensor_scalar_add(
                                yo2[:, 1, :], ypsb, pb_sb[:, i2:i2 + 1])
                            oq = nc.sync if i2 % 2 == 0 else nc.gpsimd
                            oq.dma_start(
                                yT[i2 * 128:(i2 + 1) * 128,
                                   it0 * T:it0 * T + 2 * T],
                                yo2[:].rearrange("p two t -> p (two t)"))

            # keep the warmup chain observable so nothing prunes it;
            # emitted last so it never gates a real transfer
            nc.gpsimd.dma_start(wuout[:, :], wu_keep)

    nc.compile()
    return nc


def _prep_inputs(x, kqv_w, kqv_b, proj_w, proj_b, w):
    x = np.asarray(x, np.float32)
    kqv_w = np.asarray(kqv_w, np.float32)
    kqv_b = np.asarray(kqv_b, np.float32)
    proj_w = np.asarray(proj_w, np.float32)
    proj_b = np.asarray(proj_b, np.float32)
    w = np.asarray(w, np.float32)
    f16 = np.float16
    bf16 = ml_dtypes.bfloat16

    Wk, Wq, Wv = kqv_w[0:C], kqv_w[C:2 * C], kqv_w[2 * C:3 * C]
    # pair layout: block p (256 cols) = [k_h1 k_h2 (128) | q_h1 q_h2 (128)]
    wkq = np.empty((C, 2 * C), np.float32)
    for p in range(NP):
        h1, h2 = 2 * p, 2 * p + 1
        base = p * 256
        wkq[:, base:base + 64] = Wk[h1 * 64:(h1 + 1) * 64, :].T
        wkq[:, base + 64:base + 128] = Wk[h2 * 64:(h2 + 1) * 64, :].T
        wkq[:, base + 128:base + 192] = Wq[h1 * 64:(h1 + 1) * 64, :].T
        wkq[:, base + 192:base + 256] = Wq[h2 * 64:(h2 + 1) * 64, :].T
    wv = np.ascontiguousarray(Wv.T)   # [C_in, C_out] for both kptv lhsT

    # prm pair weights: [128, p, 64]: rows 0:64 cols 0:32 = w_h1^T,
    # rows 64:128 cols 32:64 = w_h2^T
    prmw = np.zeros((128, NP * 64), np.float32)
    for p in range(NP):
        prmw[0:64, p * 64:p * 64 + 32] = w[2 * p].T
        prmw[64:128, p * 64 + 32:p * 64 + 64] = w[2 * p + 1].T
    nhalf = np.zeros((128, 64), np.float32)
    nhalf[0:64, 0:32] = -0.5
    nhalf[64:128, 32:64] = -0.5

    # biases per pair: col p = [bk_h1(64); bk_h2(64)], col NP+p = q biases
    kqb = np.zeros((128, 2 * NP), np.float32)
    for p in range(NP):
        h1, h2 = 2 * p, 2 * p + 1
        kqb[0:64, p] = kqv_b[h1 * 64:(h1 + 1) * 64]
        kqb[64:128, p] = kqv_b[h2 * 64:(h2 + 1) * 64]
        kqb[0:64, NP + p] = kqv_b[C + h1 * 64:C + (h1 + 1) * 64]
        kqb[64:128, NP + p] = kqv_b[C + h2 * 64:C + (h2 + 1) * 64]
    # v-bias folds into the proj bias: y += bv (per head-dim), so
    # out += proj_w @ bv. Exact when D >> eps or bv == 0 (true here).
    bv = kqv_b[2 * C:3 * C]
    pb_eff = proj_b + proj_w @ bv

    pwT = np.ascontiguousarray(
        proj_w.T.reshape(H, 64, C).transpose(1, 0, 2).reshape(64, H * C))
    pb = np.ascontiguousarray(pb_eff.reshape(6, 128).T)
    identb = np.eye(128, dtype=bf16)
    # ind12[h, g*128 + r] = 1 iff h == 4g + r//32 (bcast selector per quad)
    ind12 = np.zeros((12, 3 * 128), np.float32)
    for g in range(3):
        for gi in range(4):
            ind12[4 * g + gi, g * 128 + 32 * gi:g * 128 + 32 * (gi + 1)] = 1.0

    shared = {"wkq": np.ascontiguousarray(
                  wkq.reshape(6, 128, 2 * C)).astype(f16),
              "wv": wv.astype(bf16),
              "prmw": prmw.astype(f16), "nhalf": nhalf.astype(f16),
              "kqb": kqb, "pwT": pwT.astype(bf16), "pb": pb,
              "identb": identb, "ind12": ind12.astype(bf16)}
    xTb = np.ascontiguousarray(x.transpose(0, 2, 1)).astype(f16)  # [B, C, N]
    xNb = np.empty((B, N, C + 1), bf16)                  # [B, N, C | ones]
    xNb[:, :, 0:C] = x.astype(bf16)
    xNb[:, :, C] = bf16(1.0)
    return [dict(shared, xT=xTb[b], xN=xNb[b]) for b in range(B)]


def kernel(x, kqv_w, kqv_b, proj_w, proj_b, w):
    global LAST_EXEC_NS
    from concourse.bass_utils import run_bass_kernel_spmd

    if "nc" not in _CACHE:
        _CACHE["nc"] = _build()
    nc = _CACHE["nc"]

    in_maps = _prep_inputs(x, kqv_w, kqv_b, proj_w, proj_b, w)
    res = run_bass_kernel_spmd(nc, in_maps, list(range(B)), trace=TRACE)
    LAST_EXEC_NS = res.exec_time_ns
    out = np.empty((B, N, C), np.float32)
    for b in range(B):
        out[b] = np.asarray(res.results[b]["yT"]).astype(np.float32).T
    return out
